# revision 44
# baseline (speedup 1.0000x reference)
"""Trainium2 Bass kernel for bit-serial conv2d (nn_CustomConv2).

The reference's bit-serial inner loop collapses exactly to
    g(x, w) = trunc(x * w / 16)           (bits = 4)
so   out = relu(bias + sum_{i,j,c} trunc(x * w / 16)).

Since x in [0,16) and w in [-8,8), write |w| = a and decompose over a:
    trunc(x*w/16) = sum_{a=2..8} floor(x*a/16) * ([w==a] - [w==-a])
(a=1 contributes floor(x/16) = 0).  This linearizes the truncation into 7
"plane" activations A_a = floor(x*a/16) (small ints 0..7, exact in fp8 e4m3)
against {-1,0,1} masks derived from the weights.  The 7 planes (+ a
constant-1 plane carrying the bias) are packed 4-per-instruction into fp8
DoubleRow matmuls: lhsT [K=128, 2, F], rhs [K=128, 2, N], contracting 256
rows per instruction at 2 moving elements/cycle.  2 plane-pairs x 9 kernel
positions x 3 row-banks = 54 matmuls, accumulated exactly in fp32 PSUM.

Host does only sharding/padding and layout: x is shipped pre-transposed
[C, y*x] (duplicated into both partition halves) as fp8 (ints 0..15 are
exact), the weight masks are repacked into DoubleRow order with the bias
folded in as an extra mask row against the constant plane, and the [F, pix]
fp16 output is transposed back.  The data path (plane computation, conv,
bias, relu) runs on device.

Sharding: batch (4) x H-halves (2) = 8 cores, 512 output pixels per core;
masks replicated.
"""

import numpy as np

import concourse.bass as bass
import concourse.bacc as bacc
import concourse.mybir as mybir
from concourse.tile import TileContext
from concourse import bass_utils

F32 = mybir.dt.float32
BF16 = mybir.dt.bfloat16
FP16 = mybir.dt.float16
FP8 = mybir.dt.float8e4
FP8_NP = mybir.dt.np(FP8)

B, H, W, C, F = 4, 32, 32, 64, 128
KH = KW = 3
NCORES = 8
HL = H // 2          # output rows per core
YR = HL + 2          # input rows incl halo
XR = W + 2           # input cols incl pad
YX = YR * XR         # 612 spatial positions per core
XPAD = 616           # padded cols (bank3 windows read up to 614)
PIX = HL * W         # 512 output pixels per core
NPOS = KH * KW       # 9
# chunk t covers plane multipliers (2+2t, 3+2t); t=3 is (8, const-1/bias)
CHUNK_A = [(2, 3), (4, 5), (6, 7), (8, 0)]
# DoubleRow pairs: pair P contracts chunks (2P, 2P+1) in one instruction
BANK_ROWS = [(0, 8), (8, 5), (13, 2), (15, 1)]  # (row0, nrows) per bank
HSPLIT = 344         # plane column split: bank0 windows read cols < 344

MAGIC = 12582912.0   # 1.5 * 2^23: float round-to-int magic constant
OFF = -0.46875       # -15/32: fraction offset so magic round floors
N_WARM = 5           # PE p-state warmup matmuls
WARM_N = 512         # moving size of each warmup matmul


def _build_nc(n_warm=N_WARM):
    nc = bacc.Bacc(num_swdge_queues=2)
    xin = nc.dram_tensor("xin", [128, XPAD], FP8, kind="ExternalInput")
    # weight masks, DoubleRow layout [row, pos, i, f]; pair0 split for DMA
    w0a = nc.dram_tensor("w0a", [128, 5 * 2 * F], FP8, kind="ExternalInput")
    w0b = nc.dram_tensor("w0b", [128, 4 * 2 * F], FP8, kind="ExternalInput")
    w1 = nc.dram_tensor("w1", [128, NPOS * 2 * F], FP8, kind="ExternalInput")
    yout = nc.dram_tensor("yout", [128, PIX], FP16, kind="ExternalOutput")

    with TileContext(nc) as tc:
        with (
            tc.tile_pool(name="const", bufs=1) as cpool,
            tc.tile_pool(name="wp", bufs=1) as wpool,
            tc.tile_pool(name="xp", bufs=1) as xpool,
            tc.tile_pool(name="op", bufs=1) as opool,
            tc.tile_pool(name="pacc", bufs=1, space="PSUM") as paccpool,
            tc.tile_pool(name="pscr", bufs=1, space="PSUM") as pscrpool,
        ):
            # --- input DMAs head every critical path.  SP: x.  Pool: w
            # pair1 via SWDGE (bypasses the shared HWDGE generator, so its
            # descriptor gen runs in parallel and its copy lands right
            # after x's).  ACT: w pair0 in two chunks.
            xt = xpool.tile([128, XPAD], FP8, tag="xt")
            nc.sync.dma_start(out=xt[:, :], in_=xin[:, :])

            w1t = wpool.tile([128, NPOS * 2 * F], FP8, tag="w1")
            nc.gpsimd.dma_start(out=w1t[:, :], in_=w1[:, :])

            w0bt = wpool.tile([128, 4 * 2 * F], FP8, tag="w0b")
            nc.scalar.dma_start(out=w0bt[:, :], in_=w0b[:, :])
            w0at = wpool.tile([128, 5 * 2 * F], FP8, tag="w0a")
            nc.scalar.dma_start(out=w0at[:, :], in_=w0a[:, :])

            warm = cpool.tile([128, WARM_N], FP8, tag="warm")
            nc.vector.memset(warm[:, :], 0)

            # per-chunk scale vectors (chunks 0-2; chunk3's lower half uses
            # a plain float scale)
            vas = []
            for t in range(3):
                a0, a1 = CHUNK_A[t]
                va = cpool.tile([128, 1], F32, tag=f"va{t}", name=f"va{t}")
                nc.gpsimd.memset(va[0:64, :], a0 / 16.0)
                nc.gpsimd.memset(va[64:128, :], a1 / 16.0)
                vas.append(va)

            # ctx-index tiles for the output writeback preps
            idx0 = cpool.tile([128, 1], mybir.dt.int32, tag="idx0")
            nc.gpsimd.memset(idx0[:, :], 0)
            idx1 = cpool.tile([128, 1], mybir.dt.int32, tag="idx1")
            nc.gpsimd.memset(idx1[:, :], 256)

            # planes: pair tiles [128, 2*XPAD] fp8; chunk t = 2*pair + i
            xas = [xpool.tile([128, XPAD], BF16, tag=f"xa{t}", name=f"xa{t}")
                   for t in range(4)]
            pairs = [xpool.tile([128, 2 * XPAD], FP8, tag=f"pl{p}",
                                name=f"pl{p}") for p in range(2)]
            # chunk3's upper half is the constant-1 plane (bias trick)
            nc.gpsimd.memset(pairs[1][64:128, XPAD:2 * XPAD], 1.0)

            # --- PE p-state warmups: keep the tensor engine busy from ~1us
            # until the first real matmul so it reaches full clock
            for _ in range(n_warm):
                scr = pscrpool.tile([128, WARM_N], F32, tag="scr")
                nc.tensor.matmul(
                    scr[:, :], lhsT=warm[:, 0:128], rhs=warm[:, :],
                    start=True, stop=True,
                )

            # --- plane ops: floor(x*a/16) via round-to-nearest of
            # x*(a/16) - 15/32, the rounding realized by the f32
            # +/- 1.5*2^23 magic add.  Computed in column halves; pair1
            # (chunks 2,3) first since its weights arrive first.
            def op_a(t, eng, lo, hi):
                rows = slice(0, 64) if t == 3 else slice(0, 128)
                scale = 0.5 if t == 3 else vas[t][:, :]
                if eng == "act":
                    nc.scalar.activation(
                        out=xas[t][rows, lo:hi], in_=xt[rows, lo:hi],
                        func=mybir.ActivationFunctionType.Copy,
                        bias=OFF, scale=scale,
                    )
                else:
                    e = nc.vector if eng == "dve" else nc.gpsimd
                    e.tensor_scalar(
                        out=xas[t][rows, lo:hi], in0=xt[rows, lo:hi],
                        scalar1=scale, scalar2=OFF,
                        op0=mybir.AluOpType.mult, op1=mybir.AluOpType.add,
                    )

            def op_b(t, lo, hi):
                rows = slice(0, 64) if t == 3 else slice(0, 128)
                nc.vector.tensor_scalar(
                    out=pairs[t // 2][rows, (t % 2) * XPAD + lo:
                                      (t % 2) * XPAD + hi],
                    in0=xas[t][rows, lo:hi],
                    scalar1=MAGIC, scalar2=-MAGIC,
                    op0=mybir.AluOpType.add, op1=mybir.AluOpType.add,
                )

            # Ordering floors for the Tile list scheduler: it pops ready
            # instructions by (wait-floor, priority) in its own virtual
            # time, so floors must DOMINATE natural readiness (~10us) to
            # pin a strict per-engine order.  Floors are scheduler-only;
            # the real timeline still starts each op as soon as its deps
            # are satisfied.
            def at(us):
                return tc.tile_wait_until(us * 1e-3)

            # DVE: pair1 h1 chain first, then pair0 h1, then h2 op_b's.
            # The partner op_a of each DVE op_b runs on Pool/ACT in
            # parallel (Pool has no access-latency ack, so pool->DVE
            # handoffs are cheap; ACT's costs ~220ns extra).
            # DVE h1 interleave: each op_b sits >=1 slot after its op_a so
            # the ~95ns dependent-op sem gap hides under independent work
            with at(50):
                op_a(2, "dve", 0, HSPLIT)
            with at(50):
                op_a(3, "pool", 0, HSPLIT)
            with at(51):
                op_a(0, "dve", 0, HSPLIT)
            with at(51):
                op_a(1, "act", 0, HSPLIT)
            with at(52):
                op_b(2, 0, HSPLIT)
            with at(53):
                op_b(3, 0, HSPLIT)
            with at(54):
                op_b(0, 0, HSPLIT)
            with at(55):
                op_b(1, 0, HSPLIT)
            with at(51):
                op_a(2, "pool", HSPLIT, XPAD)
            with at(52):
                op_a(3, "pool", HSPLIT, XPAD)
            with at(52):
                op_a(0, "act", HSPLIT, XPAD)
            with at(53):
                op_a(1, "act", HSPLIT, XPAD)
            with at(56):
                op_b(2, HSPLIT, XPAD)
            with at(57):
                op_b(3, HSPLIT, XPAD)
            with at(58):
                op_b(0, HSPLIT, XPAD)
            with at(59):
                op_b(1, HSPLIT, XPAD)

            # --- the conv: fp8 DoubleRow matmuls, K = 2x128 rows (4 planes)
            # per instruction, N = flat window of bank rows.  Windows are
            # contiguous runs; row-crossing elements land in dead x=32,33
            # output lanes that the epilogue skips.
            pair_vs = [pairs[p][:, :].rearrange("r (i n) -> r i n", i=2)
                       for p in range(2)]
            w0a_v = w0at[:, :].rearrange("r (q i f) -> r q i f", i=2, f=F)
            w0b_v = w0bt[:, :].rearrange("r (q i f) -> r q i f", i=2, f=F)
            w1_v = w1t[:, :].rearrange("r (q i f) -> r q i f", i=2, f=F)

            accs = [paccpool.tile([128, nr * XR], F32, tag=f"acc{bk}",
                                  name=f"acc{bk}")
                    for bk, (r0, nr) in enumerate(BANK_ROWS)]
            mm_state = {bk: 0 for bk in range(len(BANK_ROWS))}

            def mm_group(bk, wv, qs, pr, poss):
                r0, nr = BANK_ROWS[bk]
                # last 2 cols of the last row-block are dead lanes no
                # window needs: trim them from every matmul's N
                nw = nr * XR - 2
                for q, p in zip(qs, poss):
                    i, j = divmod(p, KW)
                    base = (r0 + i) * XR + j
                    mm_state[bk] += 1
                    nc.tensor.matmul(
                        accs[bk][:, 0:nw],
                        lhsT=wv[:, q, :, :],
                        rhs=pair_vs[pr][:, :, base:base + nw],
                        start=(mm_state[bk] == 1),
                        stop=(mm_state[bk] == 18),
                        perf_mode=mybir.MatmulPerfMode.DoubleRow,
                    )

            def mm_p1(bk):
                mm_group(bk, w1_v, range(NPOS), 1, range(NPOS))

            def mm_p0a(bk):
                mm_group(bk, w0a_v, range(5), 0, range(5))

            def mm_p0b(bk):
                mm_group(bk, w0b_v, range(4), 0, range(5, NPOS))

            # consumption order matched to real arrival: w1 ~4.1us (SWDGE),
            # w0a ~4.6, w0b ~4.9; pair1 h1 planes ~3.9, pair0 h1 ~4.6,
            # pair1 h2 ~5.0, pair0 h2 ~5.5
            tier = [60]

            def mm_bank_groups(bk):
                with at(tier[0]):
                    mm_p1(bk)
                with at(tier[0] + 1):
                    mm_p0b(bk)
                with at(tier[0] + 2):
                    mm_p0a(bk)
                tier[0] += 3

            for bk in range(len(BANK_ROWS)):
                mm_bank_groups(bk)

            # --- epilogue: relu(acc) -> fp16 SBUF -> one DMA out.
            # bias is already in the accumulation (const plane).
            osb = opool.tile([128, PIX], FP16, tag="osb")

            def epi_relu(bk, eng):
                r0, nr = BANK_ROWS[bk]
                src = accs[bk][:, :].rearrange(
                    "p (l x) -> p l x", x=XR)[:, :, 0:W]
                dst = osb[:, r0 * W:(r0 + nr) * W].rearrange(
                    "p (l x) -> p l x", x=W)
                if eng == "act":
                    nc.scalar.activation(
                        out=dst, in_=src,
                        func=mybir.ActivationFunctionType.Relu,
                        bias=0.0, scale=1.0,
                    )
                else:
                    e = nc.vector if eng == "dve" else nc.gpsimd
                    e.tensor_scalar(
                        out=dst, in0=src, scalar1=0.0, scalar2=None,
                        op0=mybir.AluOpType.max,
                    )

            yout_v4 = yout[:, :].rearrange("p (b o n) -> b p o n",
                                           b=1, o=1)
            osbd = opool.tile([128, PIX], FP16, tag="osbd")
            nc.vector.memset(osbd[:, :], 0)

            def out_wb(lo, idx, q):
                nc.gpsimd.kv_writeback(
                    out_ap=yout_v4[:, :, :, lo:lo + 256],
                    in_ap=osbd[:, lo:lo + 256].rearrange(
                        "p (o b n) -> p o b n", o=1, b=1),
                    ctx_idxs_ap=idx[:, :],
                    prepare_only=True,
                    sem=nc.alloc_semaphore(f"out_dma_{lo}"),
                    queue_num=q,
                )

            with at(75):
                out_wb(0, idx0, 0)
            with at(76):
                out_wb(256, idx0, 1)

            with at(80):
                epi_relu(0, "act")
            with at(81):
                epi_relu(1, "act")
            with at(82):
                epi_relu(2, "act")
            with at(83):
                epi_relu(3, "dve")
            # split output: bank0's half ships early under the rest of the
            # stream; the second half is the only post-relu DMA
            with at(90):
                nc.gpsimd.trigger_dma(
                    count=None, queue_num=0,
                    signals_writable=[osb[:, 0:256]])
            with at(91):
                nc.gpsimd.trigger_dma(
                    count=None, queue_num=1,
                    signals_writable=[osb[:, 256:PIX]])
    _fix_prep_sems(nc)
    nc.finalize()
    return nc


def _fix_prep_sems(nc):
    """Point each SWDGE prep's DMA-completion sem at its DMASW lane sem.

    Tile's pass 1 ticks a DMASW lane per gen_mode==1 prep (so the final
    barrier waits `DMASW<k> >= 16`), but unlike normal Pool DMAs the lane
    sem is never attached to the prep (the `sem=` kwarg owns on_update[0]).
    Rewrite on_update[0] to the orphaned lane sem so the descriptor's
    completion bump satisfies the barrier, on hardware and in the sim.
    """
    fn = nc.m.functions[0]
    insts = [i for blk in fn.blocks for i in blk.instructions]
    updated, orphan_waits, preps = set(), {}, []
    for inst in insts:
        si = inst.sync_info
        if si is None:
            continue
        for u in si.on_update:
            if u.ant_name:
                updated.add(u.ant_name)
        for w in si.on_wait:
            if w.ant_name and w.ant_name.startswith("DMASW"):
                orphan_waits.setdefault(w.ant_name, w)
        if inst.opcode == "KVWritebackAnt" and inst.gen_mode == 1:
            preps.append(inst)
    orphans = sorted(
        (n for n in orphan_waits if n not in updated),
        key=lambda n: int(n.split("_")[0][5:]),
    )
    assert len(orphans) == len(preps), (orphans, len(preps))
    for inst, name in zip(preps, orphans):
        w = orphan_waits[name]
        u0 = inst.sync_info.on_update[0]
        u0.id = w.id
        u0.ant_name = name
    osb_ref = None
    for inst in insts:
        if inst.opcode == "Activation" and inst.outs:
            mr = inst.outs[0].memref
            if mr and mr.startswith("osb_"):
                osb_ref = mr
    assert osb_ref is not None
    for inst in preps:
        inst.ins[0].memref = osb_ref
        inst.ins[0].memsetref = osb_ref + "_set"


_NC_CACHE = {}


def _get_nc():
    if "nc" not in _NC_CACHE:
        _NC_CACHE["nc"] = _build_nc()
    return _NC_CACHE["nc"]


def make_in_maps(inputs, kernel, bias):
    """Host-side sharding + layout repacking (no arithmetic on values)."""
    x = np.asarray(inputs, dtype=np.float32)
    k = np.asarray(kernel, dtype=np.float32)
    b = np.asarray(bias, dtype=np.float32)

    # masks: wh[chunk, pos, row=(half*64+c), f] = [w==a] - [w==-a]
    wh = np.zeros((4, NPOS, 128, F), dtype=np.float32)
    kf = k.reshape(NPOS, C, F)
    for t, (a0, a1) in enumerate(CHUNK_A):
        for half, a in ((0, a0), (1, a1)):
            if a == 0:
                continue
            wh[t, :, half * 64:(half + 1) * 64, :] = (
                (kf == a).astype(np.float32) - (kf == -a).astype(np.float32)
            )
    # bias rides the constant-1 plane (chunk3 upper) at the center position
    wh[3, 4, 64, :] = b
    # DoubleRow order [row, pair, pos, i, f]
    wdr = wh.reshape(2, 2, NPOS, 128, F).transpose(3, 0, 2, 1, 4)
    wdr = np.ascontiguousarray(wdr).astype(FP8_NP)
    w0a = np.ascontiguousarray(wdr[:, 0, 0:5]).reshape(128, -1)
    w0b = np.ascontiguousarray(wdr[:, 0, 5:9]).reshape(128, -1)
    w1 = np.ascontiguousarray(wdr[:, 1]).reshape(128, -1)

    xp = np.zeros((B, H + 2, W + 2, C), dtype=np.float32)
    xp[:, 1:H + 1, 1:W + 1, :] = x
    in_maps = []
    for core in range(NCORES):
        bb, y0 = divmod(core, 2)
        slab = xp[bb, y0 * HL:y0 * HL + YR].reshape(YX, C).T  # [C, YX]
        xt = np.zeros((128, XPAD), dtype=FP8_NP)
        xt[0:64, 0:YX] = slab
        xt[64:128, 0:YX] = slab
        in_maps.append({"xin": xt, "w0a": w0a, "w0b": w0b, "w1": w1})
    return in_maps


def assemble(results):
    out = np.empty((B, H, W, F), dtype=np.float32)
    for core in range(NCORES):
        bb, y0 = divmod(core, 2)
        yc = results[core]["yout"].astype(np.float32)  # [F, PIX]
        out[bb, y0 * HL:(y0 + 1) * HL] = yc.T.reshape(HL, W, F)
    return out


def run(inputs, kernel, bias, bits, trace=False, **spmd_kwargs):
    assert int(bits) == 4, f"kernel specialized for bits=4, got {bits}"
    nc = _get_nc()
    in_maps = make_in_maps(inputs, kernel, bias)
    res = bass_utils.run_bass_kernel_spmd(
        nc, in_maps, core_ids=list(range(NCORES)), trace=trace, **spmd_kwargs
    )
    return assemble(res.results), res


def kernel(**inputs):
    out, _ = run(inputs["inputs"], inputs["kernel"], inputs["bias"],
                 inputs["bits"], trace=False)
    return out
